# revision 18
# baseline (speedup 1.0000x reference)
"""Global-attention kernel for [8, 384, 32, 32] ConvAttention on 8 trn2 cores.

Math (per reference): tokens over B*H*W = 8192 positions, C = 384 channels
split as V/K/Q of 128 each; out = softmax(Q K^T / sqrt(128)) V, re-laid as
[B, 128, H, W].

Sharding: core c owns the 1024 query tokens of batch c (token n = b*1024+hw,
so batch == contiguous token block). K/V are replicated. Each core computes
its row block of the attention entirely locally; no collectives.

On-core layout: channel-major ([d, token]) everywhere, S^T formulation.
Work is streamed in "half-tiles" [128 kv, 512 q] (one PSUM bank each):
half-tile t = (kv chunk t//2, query half t%2). Three half-tiles form a slab,
held in one of two ping-pong PSUM tiles: PE fills slab s+1 with QK matmuls
while slab s gets exp'd in a single wide instruction on ACT (or, for a
minority of slabs, a Schraudolph bit-trick exp on DVE, splitting the
elementwise load across both engines), then O^T += V^T E on PE per half-tile.

All e-side tensors are bf16 (logits reach ~|21| after scaling, so exp spans
e^-21..e^21 — beyond fp16 range). The softmax-denominator partials
accumulate as one bf16 tensor_add per slab on DVE (2x perf mode) into a
[128, 3072] accumulator whose six 512-slots alternate query halves; the
denominator fold + partition broadcast is six ones-matrix matmuls (four of
which run before the last slab), normalize = reciprocal + multiply.
"""

import math

import numpy as np

import concourse.bass as bass
import concourse.tile as tile
from concourse import bacc, mybir
from concourse.alu_op_type import AluOpType
from concourse.bass_utils import run_bass_kernel_spmd

N_CORES = 8
B, C, H, W = 8, 384, 32, 32
HW = H * W            # 1024 tokens per batch == per core
N = B * HW            # 8192 total tokens
D = 128               # key/value width
NCHUNK = N // 128     # 64 kv chunks of 128 tokens
NHALF = 2 * NCHUNK    # 128 half-tiles of [128 kv, 512 q]
NSLAB = (NHALF + 2) // 3  # 43 slabs (last one has 2 half-tiles)
SCALE = 1.0 / math.sqrt(D)
F32 = mybir.dt.float32
F32R = mybir.dt.float32r
F16 = mybir.dt.float16
BF16 = mybir.dt.bfloat16
I16 = mybir.dt.int16

# Schraudolph exp on DVE (bf16 bit trick): i16 = x*a + b truncated to int16;
# the bit pattern read as bf16 approximates exp(x*SCALE) within ~3.5%.
A7S = float((1 << 7) / math.log(2.0) * SCALE)
B7 = float(127 * (1 << 7) - 6)

# Slabs whose exp runs on DVE via the bit trick (the rest exp on ACT).
DVE_SLABS = frozenset({5, 10, 15, 20, 25, 30, 35})

N_WARMUP_MM = 14  # PE p-state warmup matmuls issued while input DMAs land


def _slab_halves(s):
    return range(3 * s, min(3 * s + 3, NHALF))


def _build_nc():
    nc = bacc.Bacc(
        "TRN2", target_bir_lowering=False, debug=False, num_devices=N_CORES
    )
    qT = nc.dram_tensor("qT", [D, HW], F32, kind="ExternalInput").ap()
    kT00 = nc.dram_tensor("kT00", [D, D], F32, kind="ExternalInput").ap()
    kT = nc.dram_tensor("kT", [D, N], F32, kind="ExternalInput").ap()
    vt16 = nc.dram_tensor("vt16", [D, N], F16, kind="ExternalInput").ap()
    oT = nc.dram_tensor("oT", [D, HW], F32, kind="ExternalOutput").ap()

    with tile.TileContext(nc) as tc:
        with (
            tc.tile_pool(name="persist", bufs=1) as persist,
            tc.tile_pool(name="etile", bufs=6) as epool,
            tc.tile_pool(name="spsum", bufs=2, space="PSUM") as spsum,
            tc.tile_pool(name="apsum", bufs=1, space="PSUM") as apsum,
        ):
            # --- SBUF persistents ---
            qT_sb = persist.tile([D, HW], F32R, tag="qT_sb")
            kT00_sb = persist.tile([D, D], F32R, tag="kT00_sb")
            kT_sb = [
                persist.tile([D, HW], F32R, tag=f"kT{i}", name=f"kT_sb{i}")
                for i in range(8)
            ]
            vt_sb = [
                persist.tile([D, HW], F16, tag=f"vt{i}", name=f"vt_sb{i}")
                for i in range(8)
            ]
            ones16 = persist.tile([D, D], BF16, tag="ones16")
            # Denominator partials: slot k (512 q cols) accumulates all
            # half-tiles with t % 6 == k; q-half of slot k is k % 2.
            rs_all = persist.tile([D, 3 * HW], BF16, tag="rs_all")
            warm_sb = persist.tile([D, 256], F32, tag="warm_sb")

            # memset only takes fp32 values; convert to bf16 on Pool (idle).
            scr32 = persist.tile([D, 3 * HW], F32, tag="scr32")
            nc.gpsimd.memset(scr32[:], 0.0)
            nc.gpsimd.tensor_copy(rs_all[:], scr32[:])
            nc.gpsimd.memset(scr32[:, 0:D], 1.0)
            nc.gpsimd.tensor_copy(ones16[:], scr32[:, 0:D])
            nc.gpsimd.memset(warm_sb[:], 0.5)

            # --- input DMAs, latency-critical pieces first ---
            nc.sync.dma_start(out=kT00_sb[:], in_=kT00[:].bitcast(F32R))
            nc.sync.dma_start(out=qT_sb[:, 0:512], in_=qT[:, 0:512].bitcast(F32R))
            nc.sync.dma_start(out=qT_sb[:, 512:1024], in_=qT[:, 512:1024].bitcast(F32R))
            for i in range(8):
                nc.sync.dma_start(
                    out=kT_sb[i][:], in_=kT[:, i * HW : (i + 1) * HW].bitcast(F32R)
                )
                nc.sync.dma_start(
                    out=vt_sb[i][:], in_=vt16[:, i * HW : (i + 1) * HW]
                )

            o_psum = apsum.tile([D, HW], F32, tag="o_psum")

            def kchunk(c):
                if c == 0:
                    return kT00_sb[:]
                blk, off = c // 8, (c % 8) * 128
                return kT_sb[blk][:, off : off + 128]

            def vchunk(c):
                blk, off = c // 8, (c % 8) * 128
                return vt_sb[blk][:, off : off + 128]

            def emit_qk_slab(s):
                width = 512 * len(_slab_halves(s))
                s_ps = spsum.tile([D, 1536], F32, tag="s", name=f"s_ps{s}")
                for i, t in enumerate(_slab_halves(s)):
                    c, h = t // 2, t % 2
                    nc.tensor.matmul(
                        s_ps[:, i * 512 : (i + 1) * 512],
                        kchunk(c),
                        qT_sb[:, h * 512 : (h + 1) * 512],
                        start=True,
                        stop=True,
                    )
                return s_ps, width

            # --- PE warmup: keep the tensor engine busy (and ramping to
            # full clock) while the first input DMAs land; results unused.
            wm_ps = spsum.tile([D, 1536], F32, tag="s", name="warm_ps")
            for i in range(N_WARMUP_MM):
                nc.tensor.matmul(
                    wm_ps[:, 0:256],
                    warm_sb[:, 0:128].bitcast(F32R),
                    warm_sb[:].bitcast(F32R),
                    start=True,
                    stop=True,
                )

            # fold order: slots 3,4,5 after slab NSLAB-2; slots 0,1,2 after
            # the final slab. rs_bc_ps comes from the spsum pool — the slot
            # rotation frees slab NSLAB-2's tile exactly when the early fold
            # runs. Track first/last per q-half for start/stop flags.
            rs_bc_ps = None
            fold_first = {0: True, 1: True}

            def emit_fold(slots, final):
                last = {h: max(k for k in slots if k % 2 == h) for h in (0, 1)}
                for k in slots:
                    h = k % 2
                    nc.tensor.matmul(
                        rs_bc_ps[:, h * 512 : (h + 1) * 512],
                        ones16[:],
                        rs_all[:, k * 512 : (k + 1) * 512],
                        start=fold_first[h],
                        stop=final and k == last[h],
                    )
                    fold_first[h] = False

            s_tiles = {0: emit_qk_slab(0)}
            for s in range(NSLAB):
                if s + 1 < NSLAB:
                    s_tiles[s + 1] = emit_qk_slab(s + 1)
                s_ps, width = s_tiles.pop(s)

                if s in DVE_SLABS:
                    e_i16 = epool.tile([D, 1536], I16, tag="e", name=f"e{s}")
                    nc.vector.tensor_scalar(
                        out=e_i16[:, 0:width],
                        in0=s_ps[:, 0:width],
                        scalar1=A7S,
                        scalar2=B7,
                        op0=AluOpType.mult,
                        op1=AluOpType.add,
                    )
                    e16 = e_i16[:].bitcast(BF16)
                else:
                    e_sb = epool.tile([D, 1536], BF16, tag="e", name=f"e{s}")
                    nc.scalar.activation(
                        e_sb[:, 0:width],
                        s_ps[:, 0:width],
                        mybir.ActivationFunctionType.Exp,
                        scale=SCALE,
                    )
                    e16 = e_sb[:]

                for i, t in enumerate(_slab_halves(s)):
                    c, h = t // 2, t % 2
                    nc.tensor.matmul(
                        o_psum[:, h * 512 : (h + 1) * 512],
                        vchunk(c),
                        e16[:, i * 512 : (i + 1) * 512],
                        start=(c == 0),
                        stop=(c == NCHUNK - 1),
                    )

                # denominator partials: one bf16 add per slab (DVE 2x mode)
                reg = (3 * s % 6) * 512
                nc.vector.tensor_add(
                    rs_all[:, reg : reg + width],
                    rs_all[:, reg : reg + width],
                    e16[:, 0:width],
                )

                if s == NSLAB - 2:
                    rs_bc_ps = spsum.tile([D, 1536], F32, tag="s", name="rs_bc_ps")
                    emit_fold([3, 4, 5], final=False)

            emit_fold([0, 1, 2], final=True)

            # --- endgame: rs_bc_ps holds the full denominator replicated
            # across partitions; normalize and store per query half.
            for h in range(2):
                sl = slice(h * 512, (h + 1) * 512)
                rec_sb = persist.tile([D, 512], F32, tag=f"rec{h}")
                nc.vector.reciprocal(rec_sb[:], rs_bc_ps[:, sl])
                o_sb = persist.tile([D, 512], F32, tag=f"osb{h}")
                nc.vector.tensor_tensor(
                    o_sb[:], o_psum[:, sl], rec_sb[:], AluOpType.mult
                )
                nc.sync.dma_start(out=oT[:, sl], in_=o_sb[:])

    nc.compile()
    return nc


_NC_CACHE = None


def _get_nc():
    global _NC_CACHE
    if _NC_CACHE is None:
        _NC_CACHE = _build_nc()
    return _NC_CACHE


def _prep_inputs(x: np.ndarray) -> list[dict]:
    x = np.ascontiguousarray(x, dtype=np.float32)
    xr = x.reshape(B, C, HW)

    # K channel-major over all tokens: kT[d, b*1024+hw] = x[b, 128+d, hw]
    kT = np.ascontiguousarray(xr[:, 128:256, :].transpose(1, 0, 2)).reshape(D, N)
    kT00 = np.ascontiguousarray(kT[:, 0:128])
    # V chunk-transposed fp16: vt[p, 128*j + v] = V[128*j + p, v]
    v_tok = np.ascontiguousarray(xr[:, 0:128, :].transpose(0, 2, 1)).reshape(N, D)
    vt16 = np.ascontiguousarray(
        v_tok.reshape(NCHUNK, 128, D).transpose(1, 0, 2)
    ).reshape(D, N).astype(np.float16)

    in_maps = []
    for c in range(N_CORES):
        qT = np.ascontiguousarray(xr[c, 256:384, :])
        in_maps.append({"qT": qT, "kT00": kT00, "kT": kT, "vt16": vt16})
    return in_maps


def kernel(x: np.ndarray) -> np.ndarray:
    assert x.shape == (B, C, H, W), x.shape
    in_maps = _prep_inputs(x)
    nc = _get_nc()
    res = run_bass_kernel_spmd(nc, in_maps, list(range(N_CORES)))

    out = np.empty((B, D, H, W), dtype=np.float32)
    for c in range(N_CORES):
        out[c] = res.results[c]["oT"].reshape(D, H, W)
    return out


# revision 21
# speedup vs baseline: 1.0385x; 1.0385x over previous
"""Global-attention kernel for [8, 384, 32, 32] ConvAttention on 8 trn2 cores.

Math (per reference): tokens over B*H*W = 8192 positions, C = 384 channels
split as V/K/Q of 128 each; out = softmax(Q K^T / sqrt(128)) V, re-laid as
[B, 128, H, W].

Sharding: core c owns the 1024 query tokens of batch c (token n = b*1024+hw,
so batch == contiguous token block). K/V are replicated. Each core computes
its row block of the attention entirely locally; no collectives.

On-core layout: channel-major ([d, token]) everywhere, S^T formulation.
Work is streamed in "half-tiles" [128 kv, 512 q] (one PSUM bank each):
half-tile t = (kv chunk t//2, query half t%2). Three half-tiles form a slab,
held in one of two ping-pong PSUM tiles: PE fills slab s+1 with QK matmuls
while slab s gets exp'd in a single wide instruction on ACT (or, for a
minority of slabs, a Schraudolph bit-trick exp on DVE, splitting the
elementwise load across both engines), then O^T += V^T E on PE per half-tile.

All e-side tensors are bf16 (logits reach ~|21| after scaling, so exp spans
e^-21..e^21 — beyond fp16 range). The softmax-denominator partials
accumulate as one bf16 tensor_add per slab on DVE (2x perf mode) into a
[128, 3072] accumulator whose six 512-slots alternate query halves; the
denominator fold + partition broadcast is six ones-matrix matmuls (four of
which run before the last slab), normalize = reciprocal + multiply.
"""

import math

import numpy as np

import concourse.bass as bass
import concourse.tile as tile
from concourse import bacc, mybir
from concourse.alu_op_type import AluOpType
from concourse.bass_utils import run_bass_kernel_spmd

N_CORES = 8
B, C, H, W = 8, 384, 32, 32
HW = H * W            # 1024 tokens per batch == per core
N = B * HW            # 8192 total tokens
D = 128               # key/value width
NCHUNK = N // 128     # 64 kv chunks of 128 tokens
NHALF = 2 * NCHUNK    # 128 half-tiles of [128 kv, 512 q]
NSLAB = (NHALF + 2) // 3  # 43 slabs (last one has 2 half-tiles)
SCALE = 1.0 / math.sqrt(D)
F32 = mybir.dt.float32
F32R = mybir.dt.float32r
F16 = mybir.dt.float16
BF16 = mybir.dt.bfloat16
I16 = mybir.dt.int16

# Schraudolph exp on DVE (bf16 bit trick): i16 = x*a + b truncated to int16;
# the bit pattern read as bf16 approximates exp(x*SCALE) within ~3.5%.
A7S = float((1 << 7) / math.log(2.0) * SCALE)
B7 = float(127 * (1 << 7) - 6)

# Slabs whose exp runs on DVE via the bit trick (the rest exp on ACT).
DVE_SLABS = frozenset({5, 10, 15, 20, 25, 30, 35})

N_WARMUP_MM = 14  # PE p-state warmup matmuls issued while input DMAs land


def _slab_halves(s):
    return range(3 * s, min(3 * s + 3, NHALF))


def _build_nc():
    nc = bacc.Bacc(
        "TRN2", target_bir_lowering=False, debug=False, num_devices=N_CORES
    )
    qT = nc.dram_tensor("qT", [D, HW], F32, kind="ExternalInput").ap()
    kT00 = nc.dram_tensor("kT00", [D, D], F32, kind="ExternalInput").ap()
    kT = nc.dram_tensor("kT", [D, N], F32, kind="ExternalInput").ap()
    vt16 = nc.dram_tensor("vt16", [D, N], F16, kind="ExternalInput").ap()
    oT = nc.dram_tensor("oT", [D, HW], F32, kind="ExternalOutput").ap()

    with tile.TileContext(nc) as tc:
        with (
            tc.tile_pool(name="persist", bufs=1) as persist,
            tc.tile_pool(name="etile", bufs=6) as epool,
            tc.tile_pool(name="spsum", bufs=2, space="PSUM") as spsum,
            tc.tile_pool(name="apsum", bufs=1, space="PSUM") as apsum,
        ):
            # --- SBUF persistents ---
            qT_sb = persist.tile([D, HW], F32R, tag="qT_sb")
            kT00_sb = persist.tile([D, D], F32R, tag="kT00_sb")
            kT_sb = [
                persist.tile([D, HW], F32R, tag=f"kT{i}", name=f"kT_sb{i}")
                for i in range(8)
            ]
            vt_sb = [
                persist.tile([D, HW], F16, tag=f"vt{i}", name=f"vt_sb{i}")
                for i in range(8)
            ]
            ones16 = persist.tile([D, D], BF16, tag="ones16")
            # Denominator partials: slot k (512 q cols) accumulates all
            # half-tiles with t % 6 == k; q-half of slot k is k % 2.
            rs_all = persist.tile([D, 3 * HW], BF16, tag="rs_all")
            warm_sb = persist.tile([D, 256], F32, tag="warm_sb")

            # memset only takes fp32 values; convert to bf16 on Pool (idle).
            # rs_all needs no init: the first touch of each region is a copy.
            scr32 = persist.tile([D, D], F32, tag="scr32")
            nc.gpsimd.memset(warm_sb[:], 0.5)
            nc.gpsimd.memset(scr32[:], 1.0)
            nc.gpsimd.tensor_copy(ones16[:], scr32[:])

            # --- input DMAs, latency-critical pieces first ---
            nc.sync.dma_start(out=kT00_sb[:], in_=kT00[:].bitcast(F32R))
            nc.sync.dma_start(out=qT_sb[:, 0:512], in_=qT[:, 0:512].bitcast(F32R))
            nc.sync.dma_start(out=qT_sb[:, 512:1024], in_=qT[:, 512:1024].bitcast(F32R))
            for i in range(8):
                nc.sync.dma_start(
                    out=kT_sb[i][:], in_=kT[:, i * HW : (i + 1) * HW].bitcast(F32R)
                )
                nc.sync.dma_start(
                    out=vt_sb[i][:], in_=vt16[:, i * HW : (i + 1) * HW]
                )

            o_psum = apsum.tile([D, HW], F32, tag="o_psum")

            def kchunk(c):
                if c == 0:
                    return kT00_sb[:]
                blk, off = c // 8, (c % 8) * 128
                return kT_sb[blk][:, off : off + 128]

            def vchunk(c):
                blk, off = c // 8, (c % 8) * 128
                return vt_sb[blk][:, off : off + 128]

            def emit_qk_slab(s):
                width = 512 * len(_slab_halves(s))
                s_ps = spsum.tile([D, 1536], F32, tag="s", name=f"s_ps{s}")
                for i, t in enumerate(_slab_halves(s)):
                    c, h = t // 2, t % 2
                    nc.tensor.matmul(
                        s_ps[:, i * 512 : (i + 1) * 512],
                        kchunk(c),
                        qT_sb[:, h * 512 : (h + 1) * 512],
                        start=True,
                        stop=True,
                    )
                return s_ps, width

            # --- PE warmup: keep the tensor engine busy (and ramping to
            # full clock) while the first input DMAs land; results unused.
            wm_ps = spsum.tile([D, 1536], F32, tag="s", name="warm_ps")
            for i in range(N_WARMUP_MM):
                nc.tensor.matmul(
                    wm_ps[:, 0:256],
                    warm_sb[:, 0:128].bitcast(F32R),
                    warm_sb[:].bitcast(F32R),
                    start=True,
                    stop=True,
                )

            # fold order: slots 3,4,5 after slab NSLAB-2; slots 0,1,2 after
            # the final slab. rs_bc_ps comes from the spsum pool — the slot
            # rotation frees slab NSLAB-2's tile exactly when the early fold
            # runs. Track first/last per q-half for start/stop flags.
            rs_bc_ps = None
            fold_first = {0: True, 1: True}

            def emit_fold(slots, final):
                last = {h: max(k for k in slots if k % 2 == h) for h in (0, 1)}
                for k in slots:
                    h = k % 2
                    nc.tensor.matmul(
                        rs_bc_ps[:, h * 512 : (h + 1) * 512],
                        ones16[:],
                        rs_all[:, k * 512 : (k + 1) * 512],
                        start=fold_first[h],
                        stop=final and k == last[h],
                    )
                    fold_first[h] = False

            def emit_exp_dve(s, s_ps, width):
                # Schraudolph exp on DVE; emitted one slab EARLY (right after
                # the slab's QK matmuls) so it overlaps ACT's exp of the
                # previous slab instead of stalling PE's in-order PV behind it.
                e_i16 = epool.tile([D, 1536], I16, tag="e", name=f"e{s}")
                nc.vector.tensor_scalar(
                    out=e_i16[:, 0:width],
                    in0=s_ps[:, 0:width],
                    scalar1=A7S,
                    scalar2=B7,
                    op0=AluOpType.mult,
                    op1=AluOpType.add,
                )
                return e_i16[:].bitcast(BF16)

            s_tiles = {0: emit_qk_slab(0)}
            e_early = {}
            for s in range(NSLAB):
                if s + 1 < NSLAB:
                    s_tiles[s + 1] = emit_qk_slab(s + 1)
                    if s + 1 in DVE_SLABS:
                        nxt_ps, nxt_w = s_tiles[s + 1]
                        e_early[s + 1] = emit_exp_dve(s + 1, nxt_ps, nxt_w)
                s_ps, width = s_tiles.pop(s)

                if s in DVE_SLABS:
                    e16 = e_early.pop(s) if s in e_early else emit_exp_dve(
                        s, s_ps, width
                    )
                else:
                    e_sb = epool.tile([D, 1536], BF16, tag="e", name=f"e{s}")
                    nc.scalar.activation(
                        e_sb[:, 0:width],
                        s_ps[:, 0:width],
                        mybir.ActivationFunctionType.Exp,
                        scale=SCALE,
                    )
                    e16 = e_sb[:]

                for i, t in enumerate(_slab_halves(s)):
                    c, h = t // 2, t % 2
                    nc.tensor.matmul(
                        o_psum[:, h * 512 : (h + 1) * 512],
                        vchunk(c),
                        e16[:, i * 512 : (i + 1) * 512],
                        start=(c == 0),
                        stop=(c == NCHUNK - 1),
                    )

                # denominator partials: one bf16 add per slab (DVE 2x mode);
                # the first touch of each region is a copy (4x) — no init.
                reg = (3 * s % 6) * 512
                if s <= 1:
                    nc.vector.tensor_copy(
                        rs_all[:, reg : reg + width], e16[:, 0:width]
                    )
                else:
                    nc.vector.tensor_add(
                        rs_all[:, reg : reg + width],
                        rs_all[:, reg : reg + width],
                        e16[:, 0:width],
                    )

                if s == NSLAB - 2:
                    rs_bc_ps = spsum.tile([D, 1536], F32, tag="s", name="rs_bc_ps")
                    emit_fold([3, 4, 5], final=False)

            emit_fold([0, 1, 2], final=True)

            # --- endgame: rs_bc_ps holds the full denominator replicated
            # across partitions; normalize and store per query half.
            for h in range(2):
                sl = slice(h * 512, (h + 1) * 512)
                rec_sb = persist.tile([D, 512], F32, tag=f"rec{h}")
                nc.vector.reciprocal(rec_sb[:], rs_bc_ps[:, sl])
                o_sb = persist.tile([D, 512], F32, tag=f"osb{h}")
                nc.vector.tensor_tensor(
                    o_sb[:], o_psum[:, sl], rec_sb[:], AluOpType.mult
                )
                nc.sync.dma_start(out=oT[:, sl], in_=o_sb[:])

    nc.compile()
    return nc


_NC_CACHE = None


def _get_nc():
    global _NC_CACHE
    if _NC_CACHE is None:
        _NC_CACHE = _build_nc()
    return _NC_CACHE


def _prep_inputs(x: np.ndarray) -> list[dict]:
    x = np.ascontiguousarray(x, dtype=np.float32)
    xr = x.reshape(B, C, HW)

    # K channel-major over all tokens: kT[d, b*1024+hw] = x[b, 128+d, hw]
    kT = np.ascontiguousarray(xr[:, 128:256, :].transpose(1, 0, 2)).reshape(D, N)
    kT00 = np.ascontiguousarray(kT[:, 0:128])
    # V chunk-transposed fp16: vt[p, 128*j + v] = V[128*j + p, v]
    v_tok = np.ascontiguousarray(xr[:, 0:128, :].transpose(0, 2, 1)).reshape(N, D)
    vt16 = np.ascontiguousarray(
        v_tok.reshape(NCHUNK, 128, D).transpose(1, 0, 2)
    ).reshape(D, N).astype(np.float16)

    in_maps = []
    for c in range(N_CORES):
        qT = np.ascontiguousarray(xr[c, 256:384, :])
        in_maps.append({"qT": qT, "kT00": kT00, "kT": kT, "vt16": vt16})
    return in_maps


def kernel(x: np.ndarray) -> np.ndarray:
    assert x.shape == (B, C, H, W), x.shape
    in_maps = _prep_inputs(x)
    nc = _get_nc()
    res = run_bass_kernel_spmd(nc, in_maps, list(range(N_CORES)))

    out = np.empty((B, D, H, W), dtype=np.float32)
    for c in range(N_CORES):
        out[c] = res.results[c]["oT"].reshape(D, H, W)
    return out


# revision 22
# speedup vs baseline: 1.1183x; 1.0768x over previous
"""Global-attention kernel for [8, 384, 32, 32] ConvAttention on 8 trn2 cores.

Math (per reference): tokens over B*H*W = 8192 positions, C = 384 channels
split as V/K/Q of 128 each; out = softmax(Q K^T / sqrt(128)) V, re-laid as
[B, 128, H, W].

Sharding: core c owns the 1024 query tokens of batch c (token n = b*1024+hw,
so batch == contiguous token block). K/V are replicated. Each core computes
its row block of the attention entirely locally; no collectives.

On-core layout: channel-major ([d, token]) everywhere, S^T formulation:
for each kv chunk j (128 tokens), S^T_j = K_j^T Q in PSUM (3 rotating
tiles), exp on ACT (or a Schraudolph bit-trick exp on DVE for a minority of
chunks, splitting the elementwise load across both engines; those are
emitted right after their QK so they overlap ACT's exp of earlier chunks),
then O^T += V_j^T E_j on PE.

All e-side tensors are bf16 (logits reach ~|21| after scaling, so exp spans
e^-21..e^21 — beyond fp16 range). Softmax-denominator partials accumulate
as one bf16 tensor_add per chunk on DVE (2x perf mode) into a [128, 2048]
accumulator split by chunk parity; each parity region is folded (with
partition broadcast) by ones-matrix matmuls as soon as its last chunk is
summed, and the final chunk's e folds directly so the tail chain is short.
The last chunk's exp is split ACT/DVE by query half for the same reason.
Normalize = reciprocal + multiply per query half.
"""

import math

import numpy as np

import concourse.bass as bass
import concourse.tile as tile
from concourse import bacc, mybir
from concourse.alu_op_type import AluOpType
from concourse.bass_utils import run_bass_kernel_spmd

N_CORES = 8
B, C, H, W = 8, 384, 32, 32
HW = H * W            # 1024 tokens per batch == per core
N = B * HW            # 8192 total tokens
D = 128               # key/value width
NCHUNK = N // 128     # 64 kv chunks of 128 tokens
SCALE = 1.0 / math.sqrt(D)
F32 = mybir.dt.float32
F32R = mybir.dt.float32r
F16 = mybir.dt.float16
BF16 = mybir.dt.bfloat16
I16 = mybir.dt.int16

# Schraudolph exp on DVE (bf16 bit trick): i16 = x*a + b truncated to int16;
# the bit pattern read as bf16 approximates exp(x*SCALE) within ~3.5%.
A7S = float((1 << 7) / math.log(2.0) * SCALE)
B7 = float(127 * (1 << 7) - 6)

# Chunks whose exp runs on DVE via the bit trick (the rest exp on ACT).
# Kept clear of the final chunks, which sit on the drain-critical path.
DVE_CHUNKS = frozenset(c for c in range(NCHUNK - 2) if c % 6 == 2)  # 10

N_WARMUP_MM = 14  # PE p-state warmup matmuls issued while input DMAs land


def _build_nc():
    nc = bacc.Bacc(
        "TRN2", target_bir_lowering=False, debug=False, num_devices=N_CORES
    )
    qT = nc.dram_tensor("qT", [D, HW], F32, kind="ExternalInput").ap()
    kT00 = nc.dram_tensor("kT00", [D, D], F32, kind="ExternalInput").ap()
    kT = nc.dram_tensor("kT", [D, N], F32, kind="ExternalInput").ap()
    vt16 = nc.dram_tensor("vt16", [D, N], F16, kind="ExternalInput").ap()
    oT = nc.dram_tensor("oT", [D, HW], F32, kind="ExternalOutput").ap()

    with tile.TileContext(nc) as tc:
        with (
            tc.tile_pool(name="persist", bufs=1) as persist,
            tc.tile_pool(name="etile", bufs=8) as epool,
            tc.tile_pool(name="spsum", bufs=3, space="PSUM") as spsum,
            tc.tile_pool(name="apsum", bufs=1, space="PSUM") as apsum,
        ):
            # --- SBUF persistents ---
            qT_sb = persist.tile([D, HW], F32R, tag="qT_sb")
            kT00_sb = persist.tile([D, D], F32R, tag="kT00_sb")
            kT_sb = [
                persist.tile([D, HW], F32R, tag=f"kT{i}", name=f"kT_sb{i}")
                for i in range(8)
            ]
            vt_sb = [
                persist.tile([D, HW], F16, tag=f"vt{i}", name=f"vt_sb{i}")
                for i in range(8)
            ]
            ones16 = persist.tile([D, D], BF16, tag="ones16")
            # Denominator partials, split by chunk parity (region 0: even
            # chunks, region 1: odd) so each region folds early.
            rs2 = persist.tile([D, 2 * HW], BF16, tag="rs2")
            warm_sb = persist.tile([D, 256], F32, tag="warm_sb")

            # Cheap init first so PE warmup starts immediately; rs2 needs no
            # init (first touch of each region is a copy).
            scr32 = persist.tile([D, D], F32, tag="scr32")
            nc.gpsimd.memset(warm_sb[:], 0.5)
            nc.gpsimd.memset(scr32[:], 1.0)
            nc.gpsimd.tensor_copy(ones16[:], scr32[:])

            # --- input DMAs, latency-critical pieces first ---
            nc.sync.dma_start(out=kT00_sb[:], in_=kT00[:].bitcast(F32R))
            nc.sync.dma_start(out=qT_sb[:, 0:512], in_=qT[:, 0:512].bitcast(F32R))
            nc.sync.dma_start(out=qT_sb[:, 512:1024], in_=qT[:, 512:1024].bitcast(F32R))
            for i in range(8):
                nc.sync.dma_start(
                    out=kT_sb[i][:], in_=kT[:, i * HW : (i + 1) * HW].bitcast(F32R)
                )
                nc.sync.dma_start(
                    out=vt_sb[i][:], in_=vt16[:, i * HW : (i + 1) * HW]
                )

            # --- PE warmup: keep the tensor engine busy (and ramping to
            # full clock) while the first input DMAs land; results unused.
            wm_ps = spsum.tile([D, HW], F32, tag="s", name="warm_ps")
            for i in range(N_WARMUP_MM):
                nc.tensor.matmul(
                    wm_ps[:, 0:256],
                    warm_sb[:, 0:128].bitcast(F32R),
                    warm_sb[:].bitcast(F32R),
                    start=True,
                    stop=True,
                )

            o_psum = apsum.tile([D, HW], F32, tag="o_psum")

            def kchunk(c):
                if c == 0:
                    return kT00_sb[:]
                blk, off = c // 8, (c % 8) * 128
                return kT_sb[blk][:, off : off + 128]

            def vchunk(c):
                blk, off = c // 8, (c % 8) * 128
                return vt_sb[blk][:, off : off + 128]

            def emit_qk(c):
                s_ps = spsum.tile([D, HW], F32, tag="s", name=f"s_ps{c}")
                for h in range(2):
                    nc.tensor.matmul(
                        s_ps[:, h * 512 : (h + 1) * 512],
                        kchunk(c),
                        qT_sb[:, h * 512 : (h + 1) * 512],
                        start=True,
                        stop=True,
                    )
                return s_ps

            def emit_exp_dve(c, s_ps, sl=slice(0, HW)):
                e_i16 = epool.tile([D, HW], I16, tag="e", name=f"e{c}")
                nc.vector.tensor_scalar(
                    out=e_i16[:, sl],
                    in0=s_ps[:, sl],
                    scalar1=A7S,
                    scalar2=B7,
                    op0=AluOpType.mult,
                    op1=AluOpType.add,
                )
                return e_i16

            # rs_bc_ps is allocated from the spsum pool near the end (the
            # rotation frees a slot exactly when the first fold runs).
            rs_bc_ps = None
            fold_state = {0: True, 1: True}  # per-q-half "is first matmul"

            def emit_fold(src, final):
                for h in range(2):
                    nc.tensor.matmul(
                        rs_bc_ps[:, h * 512 : (h + 1) * 512],
                        ones16[:],
                        src[:, h * 512 : (h + 1) * 512],
                        start=fold_state[h],
                        stop=final,
                    )
                    fold_state[h] = False

            # Software-pipelined two chunks ahead (3 PSUM S-slots). DVE-exp
            # chunks are emitted right after their QK.
            s_tiles = {0: emit_qk(0), 1: emit_qk(1)}
            e_early = {}
            for c in range(NCHUNK):
                if c + 2 < NCHUNK:
                    s_tiles[c + 2] = emit_qk(c + 2)
                    if c + 2 in DVE_CHUNKS:
                        e_early[c + 2] = emit_exp_dve(c + 2, s_tiles[c + 2])
                s_ps = s_tiles.pop(c)

                if c in DVE_CHUNKS:
                    e16 = e_early.pop(c)[:].bitcast(BF16)
                elif c == NCHUNK - 1:
                    # Last chunk: split the exp across ACT (half 0) and DVE
                    # (half 1, bit trick) to shorten the drain chain.
                    e_sb = epool.tile([D, HW], BF16, tag="e", name=f"e{c}")
                    nc.scalar.activation(
                        e_sb[:, 0:512],
                        s_ps[:, 0:512],
                        mybir.ActivationFunctionType.Exp,
                        scale=SCALE,
                    )
                    nc.vector.tensor_scalar(
                        out=e_sb[:, 512:1024].bitcast(I16),
                        in0=s_ps[:, 512:1024],
                        scalar1=A7S,
                        scalar2=B7,
                        op0=AluOpType.mult,
                        op1=AluOpType.add,
                    )
                    e16 = e_sb[:]
                else:
                    e_sb = epool.tile([D, HW], BF16, tag="e", name=f"e{c}")
                    nc.scalar.activation(
                        e_sb[:],
                        s_ps[:],
                        mybir.ActivationFunctionType.Exp,
                        scale=SCALE,
                    )
                    e16 = e_sb[:]

                for h in range(2):
                    nc.tensor.matmul(
                        o_psum[:, h * 512 : (h + 1) * 512],
                        vchunk(c),
                        e16[:, h * 512 : (h + 1) * 512],
                        start=(c == 0),
                        stop=(c == NCHUNK - 1),
                    )

                # Denominator partials (skip the last chunk: its e folds
                # directly on PE). First touch of each region is a copy.
                if c < NCHUNK - 1:
                    reg = (c % 2) * HW
                    region = rs2[:, reg : reg + HW]
                    if c < 2:
                        nc.vector.tensor_copy(region, e16)
                    else:
                        nc.vector.tensor_add(region, region, e16)

                if c == NCHUNK - 3:
                    # chunk 61 was the last odd rs2 contributor
                    rs_bc_ps = spsum.tile([D, HW], F32, tag="s", name="rs_bc_ps")
                    emit_fold(rs2[:, HW : 2 * HW], final=False)
                elif c == NCHUNK - 2:
                    # chunk 62 was the last even rs2 contributor
                    emit_fold(rs2[:, 0:HW], final=False)
                elif c == NCHUNK - 1:
                    emit_fold(e16, final=True)

            # --- endgame: rs_bc_ps holds the full denominator replicated
            # across partitions; normalize and store per query half.
            for h in range(2):
                sl = slice(h * 512, (h + 1) * 512)
                rec_sb = persist.tile([D, 512], F32, tag=f"rec{h}")
                nc.vector.reciprocal(rec_sb[:], rs_bc_ps[:, sl])
                o_sb = persist.tile([D, 512], F32, tag=f"osb{h}")
                nc.vector.tensor_tensor(
                    o_sb[:], o_psum[:, sl], rec_sb[:], AluOpType.mult
                )
                nc.sync.dma_start(out=oT[:, sl], in_=o_sb[:])

    nc.compile()
    return nc


_NC_CACHE = None


def _get_nc():
    global _NC_CACHE
    if _NC_CACHE is None:
        _NC_CACHE = _build_nc()
    return _NC_CACHE


def _prep_inputs(x: np.ndarray) -> list[dict]:
    x = np.ascontiguousarray(x, dtype=np.float32)
    xr = x.reshape(B, C, HW)

    # K channel-major over all tokens: kT[d, b*1024+hw] = x[b, 128+d, hw]
    kT = np.ascontiguousarray(xr[:, 128:256, :].transpose(1, 0, 2)).reshape(D, N)
    kT00 = np.ascontiguousarray(kT[:, 0:128])
    # V chunk-transposed fp16: vt[p, 128*j + v] = V[128*j + p, v]
    v_tok = np.ascontiguousarray(xr[:, 0:128, :].transpose(0, 2, 1)).reshape(N, D)
    vt16 = np.ascontiguousarray(
        v_tok.reshape(NCHUNK, 128, D).transpose(1, 0, 2)
    ).reshape(D, N).astype(np.float16)

    in_maps = []
    for c in range(N_CORES):
        qT = np.ascontiguousarray(xr[c, 256:384, :])
        in_maps.append({"qT": qT, "kT00": kT00, "kT": kT, "vt16": vt16})
    return in_maps


def kernel(x: np.ndarray) -> np.ndarray:
    assert x.shape == (B, C, H, W), x.shape
    in_maps = _prep_inputs(x)
    nc = _get_nc()
    res = run_bass_kernel_spmd(nc, in_maps, list(range(N_CORES)))

    out = np.empty((B, D, H, W), dtype=np.float32)
    for c in range(N_CORES):
        out[c] = res.results[c]["oT"].reshape(D, H, W)
    return out
